# revision 14
# baseline (speedup 1.0000x reference)
"""DiagonalLinear (Toeplitz linear layer) Trainium2 kernel.

y[b,s,o] = sum_i x[b,s,i] * W[o,i] + bias[o],  W[o,i] = vals[(i-o) mod (OUT+IN-1)]
x: [4, 2048, 4096] f32, diagonals: [8191] f32, bias: [4096] f32.

Strategy (8 NeuronCores, data parallel over the 8192 flattened rows):
 - Each core computes 1024 rows: y_c = x_c @ W.T + bias.
 - The Toeplitz weight is never materialized (64 MB); instead each core
   holds a compact SBUF expansion rv[p, u] = vals[(3968 + p - u) mod 8191]
   (bf16) from which every [128k x 512n] weight tile is a plain free-dim
   slice: rhs(kc, nc) = rv[:, 3968 + 512*nc - 128*kc :+512].
 - x is fed pre-transposed per core (xT [4096, 1024] bf16, host-side
   layout prep during sharding) so lhsT chunks [128k, 128m] load
   contiguously.
 - Mixed precision split-K: contraction k in [512, 4096) runs bf16
   (215-216 ns per N=512 matmul = the streaming floor; bf16 LDWEIGHTS
   at 97 ns hides fully, unlike f32r's 224 ns which paced the original
   baseline at 272 ns/matmul). k in [0, 512) runs fp8 e4m3 with
   perf_mode=DoubleRow (2 rows/cycle): 2 DR matmuls of K=256 replace 4
   bf16 matmuls per output tile, saving ~24 us. Measured rel err of the
   hybrid vs f64 is ~1.4e-2 (gate 2e-2); fp8 on all K would be ~4e-2.
 - Loop (mc, ncol, kc): one PSUM chain at a time, drained (DVE bias add,
   f32->bf16) right when it stops, so drains/stores overlap the matmul
   stream and the kernel tail is a single drain + store.
 - 10 dummy warm-up matmuls run during the initial DMA wait so the PE
   HAM clock-gate reaches 2.4 GHz about when real work starts.
 - y is stored bf16 and upcast to f32 on the host.
"""

import numpy as np
import ml_dtypes

import bass_rust
import concourse.bass as bass
import concourse.mybir as mybir
import concourse.tile as tile
from concourse.bass_utils import run_bass_kernel_spmd

BF16 = ml_dtypes.bfloat16
E4M3 = ml_dtypes.float8_e4m3fn

IN_F = 4096
OUT_F = 4096
NVALS = OUT_F + IN_F - 1  # 8191
B, S = 4, 2048
ROWS = B * S              # 8192
N_CORES = 8
M_PER_CORE = ROWS // N_CORES  # 1024

MT = 128                  # m tile (PSUM partition dim)
NT = 512                  # n tile (one PSUM bank of fp32)
KT = 128                  # k tile (PE contraction dim)
N_MC = M_PER_CORE // MT   # 8
N_NC = OUT_F // NT        # 8
N_KC = IN_F // KT         # 32

N_F8KC = 10               # k-tiles [0, N_F8KC) run fp8 DoubleRow
N_DR = N_F8KC // 2        # DR matmuls per chain (each covers K=256)
N_BKC = N_KC - N_F8KC     # bf16 k-tiles per chain (kc in [N_F8KC, N_KC))

# bf16 rv: cols c = 3968 + 512*ncol - 128*kc for kc in [N_F8KC, N_KC)
RV_C0 = (N_KC - 1) * KT   # 3968
RV_F = RV_C0 - N_F8KC * KT + (N_NC - 1) * NT + NT  # 7552

# fp8 rv8[p, i, u] = vals[(C8 + 2p + i - u) mod 8191]; DR matmul t of
# output column block n0 streams rv8[:, :, s:s+512] with s = C8 - 256t + n0
C8 = 256 * (N_DR - 1)
RV8_F = C8 + (N_NC - 1) * NT + NT  # 4864

N_WARMUP_MM = 10          # dummy matmuls to lift the HAM clock gate

_COMPILED = None


def _legalize_single_wait(nc):
    """This walrus build encodes at most one sync-wait per instruction;
    move extra waits onto carrier NoOps on the same engine."""
    for f in nc.m.functions:
        for blk in f.blocks:
            insts = blk.instructions
            new = []
            changed = False
            for inst in insts:
                si = inst.sync_info
                if si is not None and si.on_wait is not None and len(si.on_wait) > 1:
                    waits = list(si.on_wait)
                    for w in waits[:-1]:
                        nop = mybir.InstNoOp(name=f"I-waitsplit-{nc.next_id()}")
                        nop.engine = inst.engine
                        nop.sync_info = bass_rust.SyncInfo(on_wait=[w], on_update=[])
                        new.append(nop)
                    inst.sync_info = bass_rust.SyncInfo(
                        on_wait=[waits[-1]], on_update=si.on_update
                    )
                    changed = True
                new.append(inst)
            if changed:
                blk.instructions = new


def build_nc():
    f32 = mybir.dt.float32
    bf16 = mybir.dt.bfloat16
    fp8 = mybir.dt.float8e4
    nc = bass.Bass()
    xT = nc.dram_tensor("xT", [IN_F, M_PER_CORE], bf16, kind="ExternalInput")
    xt8d = nc.dram_tensor(
        "xt8", [128, N_DR, 2, M_PER_CORE], fp8, kind="ExternalInput"
    )
    rv = nc.dram_tensor("rv", [128, RV_F], bf16, kind="ExternalInput")
    rv8d = nc.dram_tensor("rv8", [128, 2, RV8_F], fp8, kind="ExternalInput")
    bias_rep = nc.dram_tensor("bias_rep", [128, OUT_F], bf16, kind="ExternalInput")
    y = nc.dram_tensor("y", [M_PER_CORE, OUT_F], bf16, kind="ExternalOutput")

    # [128, N_BKC, M]; position q holds k-chunk kc = q + N_F8KC
    xT_r = xT.rearrange("(kc p) m -> p kc m", p=128)[:, N_F8KC:, :]

    with tile.TileContext(nc) as tc:
        with (
            tc.tile_pool(name="const", bufs=1) as cpool,
            tc.tile_pool(name="xp", bufs=3) as xpool,
            tc.tile_pool(name="op", bufs=8) as opool,
            tc.tile_pool(name="pp", bufs=8, space="PSUM") as ppool,
        ):
            # PE warm-up: the HAM clock gate keeps the PE at 1.2 GHz until
            # it has been busy ~3.4us. Run dummy matmuls on a memset tile
            # during the initial DMA wait so the clock is (mostly) up when
            # real matmuls start. The junk PSUM tile shares the 8-bank
            # 'acc' rotation; every real chain's start=True clear makes
            # that safe.
            junk_sb = cpool.tile([128, MT], bf16, name="junk_sb")
            nc.vector.memset(junk_sb, 0.0)
            junk_ps = ppool.tile([MT, MT], f32, tag="acc", name="junk_ps")
            for _ in range(N_WARMUP_MM):
                nc.tensor.matmul(junk_ps, junk_sb, junk_sb, start=True, stop=True)

            # Startup-critical path: matmul #0 (ncol=0, kc=31) needs only rv
            # cols [0,512) (128 KB) and xt chunk 31 (32 KB); the first chain
            # then consumes one more xt chunk and 128 more rv cols per
            # (bf16) step, reaching its two fp8 DR matmuls ~6 us in and the
            # first drain (bias) after that. Issue DMAs in consumption order.
            xt_first = xpool.tile([128, N_BKC, MT], bf16, tag="xt")
            xt8_first = xpool.tile([128, N_DR, 2, MT], fp8, tag="xt8")
            rv_sb = cpool.tile([128, RV_F], bf16)
            rv8_sb = cpool.tile([128, 2, RV8_F], fp8)
            bias_sb = cpool.tile([128, OUT_F], bf16)

            def rv_load(c0, c1):
                nc.sync.dma_start(out=rv_sb[:, c0:c1], in_=rv[:, c0:c1])

            def rv8_load(c0, c1):
                nc.sync.dma_start(
                    out=rv8_sb[:, :, c0:c1], in_=rv8d[:, :, c0:c1]
                )

            def xt_load(q0, q1):
                nc.sync.dma_start(
                    out=xt_first[:, q0:q1, :], in_=xT_r[:, q0:q1, 0:MT]
                )

            rv_load(0, 512)
            xt_load(N_BKC - 1, N_BKC)
            xt_load(N_BKC - 4, N_BKC - 1)
            rv_load(512, 1024)
            rv_load(1024, 2048)
            xt_load(N_BKC - 8, N_BKC - 4)
            rv8_load(0, 1536)
            nc.sync.dma_start(
                out=xt8_first, in_=xt8d[:, :, :, 0:MT]
            )
            rv_load(2048, 3072)
            xt_load(N_BKC - 16, N_BKC - 8)
            rv_load(3072, 4096)
            xt_load(0, N_BKC - 16)
            rv_load(4096, 5632)
            rv_load(5632, RV_F)
            rv8_load(1536, 3072)
            nc.sync.dma_start(out=bias_sb, in_=bias_rep[:, :])
            rv8_load(3072, RV8_F)

            for mc in range(N_MC):
                m0 = mc * MT
                if mc == 0:
                    xt_sb = xt_first
                    xt8_sb = xt8_first
                else:
                    xt_sb = xpool.tile([128, N_BKC, MT], bf16, tag="xt")
                    nc.sync.dma_start(out=xt_sb, in_=xT_r[:, :, m0 : m0 + MT])
                    xt8_sb = xpool.tile([128, N_DR, 2, MT], fp8, tag="xt8")
                    nc.sync.dma_start(
                        out=xt8_sb, in_=xt8d[:, :, :, m0 : m0 + MT]
                    )
                for ncol in range(N_NC):
                    n0 = ncol * NT
                    acc = ppool.tile([MT, NT], f32, tag="acc", name="acc")
                    for kk, kc in enumerate(reversed(range(N_F8KC, N_KC))):
                        c = RV_C0 + n0 - kc * KT
                        nc.tensor.matmul(
                            acc,
                            xt_sb[:, kc - N_F8KC, :],
                            rv_sb[:, c : c + NT],
                            start=(kk == 0),
                            stop=False,
                        )
                    for t in range(N_DR):
                        s = C8 - 256 * t + n0
                        nc.tensor.matmul(
                            acc,
                            xt8_sb[:, t],
                            rv8_sb[:, :, s : s + NT],
                            start=False,
                            stop=(t == N_DR - 1),
                            perf_mode=mybir.MatmulPerfMode.DoubleRow,
                        )
                    out_sb = opool.tile([MT, NT], bf16, tag="out")
                    nc.vector.tensor_add(
                        out_sb, acc, bias_sb[:, n0 : n0 + NT]
                    )
                    nc.sync.dma_start(
                        out=y[m0 : m0 + MT, n0 : n0 + NT], in_=out_sb
                    )
    _legalize_single_wait(nc)
    return nc


def _prep_shared(diagonals, bias):
    vals = np.concatenate([diagonals[OUT_F - 1 :], diagonals[: OUT_F - 1]])
    p = np.arange(128)[:, None]
    u = np.arange(RV_F)[None, :]
    rv = np.ascontiguousarray(vals[(RV_C0 + p - u) % NVALS].astype(BF16))
    p3 = np.arange(128)[:, None, None]
    i3 = np.arange(2)[None, :, None]
    u3 = np.arange(RV8_F)[None, None, :]
    rv8 = np.ascontiguousarray(
        vals[(C8 + 2 * p3 + i3 - u3) % NVALS].astype(E4M3)
    )
    bias_rep = np.ascontiguousarray(
        np.broadcast_to(bias.astype(BF16), (128, OUT_F))
    )
    return rv, rv8, bias_rep


def _prep_in_maps(x, diagonals, bias):
    """Host-side layout prep: per-core input dicts for run_bass_kernel_spmd."""
    x = np.asarray(x, dtype=np.float32)
    diagonals = np.asarray(diagonals, dtype=np.float32)
    bias = np.asarray(bias, dtype=np.float32)

    rv, rv8, bias_rep = _prep_shared(diagonals, bias)
    x2 = x.reshape(ROWS, IN_F)
    x2t = x2.T  # [IN_F, ROWS] view
    xb = x2t.astype(BF16)
    # xt8[ki, t, ko, m] = x2t[k, m] quantized e4m3, k = 256t + 2ki + ko
    ki = np.arange(128)[:, None, None]
    t = np.arange(N_DR)[None, :, None]
    ko = np.arange(2)[None, None, :]
    kidx = 256 * t + 2 * ki + ko  # [128, N_DR, 2]
    x8 = x2t[kidx, :].astype(E4M3)  # [128, N_DR, 2, ROWS]
    in_maps = []
    for c in range(N_CORES):
        msl = slice(c * M_PER_CORE, (c + 1) * M_PER_CORE)
        in_maps.append(
            {
                "xT": np.ascontiguousarray(xb[:, msl]),
                "xt8": np.ascontiguousarray(x8[:, :, :, msl]),
                "rv": rv,
                "rv8": rv8,
                "bias_rep": bias_rep,
            }
        )
    return in_maps


def kernel(x, diagonals, bias):
    global _COMPILED
    if _COMPILED is None:
        _COMPILED = build_nc()
    nc = _COMPILED

    in_maps = _prep_in_maps(x, diagonals, bias)
    res = run_bass_kernel_spmd(nc, in_maps, core_ids=list(range(N_CORES)))
    y = np.concatenate(
        [res.results[c]["y"].astype(np.float32) for c in range(N_CORES)],
        axis=0,
    )
    return y.reshape(B, S, OUT_F)


# revision 15
# speedup vs baseline: 1.1545x; 1.1545x over previous
"""DiagonalLinear (Toeplitz linear layer) Trainium2 kernel.

y[b,s,o] = sum_i x[b,s,i] * W[o,i] + bias[o],  W[o,i] = vals[(i-o) mod (OUT+IN-1)]
x: [4, 2048, 4096] f32, diagonals: [8191] f32, bias: [4096] f32.

Strategy (8 NeuronCores, data parallel over the 8192 flattened rows):
 - Each core computes 1024 rows: y_c = x_c @ W.T + bias.
 - The Toeplitz weight is never materialized (64 MB); instead each core
   holds a compact SBUF expansion rv[p, u] = vals[(3968 + p - u) mod 8191]
   (bf16) from which every [128k x 512n] weight tile is a plain free-dim
   slice: rhs(kc, nc) = rv[:, 3968 + 512*nc - 128*kc :+512].
 - x is fed pre-transposed per core (xT [4096, 1024] bf16, host-side
   layout prep during sharding) so lhsT chunks [128k, 128m] load
   contiguously.
 - Mixed precision split-K: contraction k in [512, 4096) runs bf16
   (215-216 ns per N=512 matmul = the streaming floor; bf16 LDWEIGHTS
   at 97 ns hides fully, unlike f32r's 224 ns which paced the original
   baseline at 272 ns/matmul). k in [0, 512) runs fp8 e4m3 with
   perf_mode=DoubleRow (2 rows/cycle): 2 DR matmuls of K=256 replace 4
   bf16 matmuls per output tile, saving ~24 us. Measured rel err of the
   hybrid vs f64 is ~1.4e-2 (gate 2e-2); fp8 on all K would be ~4e-2.
 - Loop (mc, ncol, kc): one PSUM chain at a time, drained (DVE bias add,
   f32->bf16) right when it stops, so drains/stores overlap the matmul
   stream and the kernel tail is a single drain + store.
 - 10 dummy warm-up matmuls run during the initial DMA wait so the PE
   HAM clock-gate reaches 2.4 GHz about when real work starts.
 - y is stored bf16 and upcast to f32 on the host.
"""

import numpy as np
import ml_dtypes

import bass_rust
import concourse.bass as bass
import concourse.mybir as mybir
import concourse.tile as tile
from concourse.bass_utils import run_bass_kernel_spmd

BF16 = ml_dtypes.bfloat16
E4M3 = ml_dtypes.float8_e4m3fn

IN_F = 4096
OUT_F = 4096
NVALS = OUT_F + IN_F - 1  # 8191
B, S = 4, 2048
ROWS = B * S              # 8192
N_CORES = 8
M_PER_CORE = ROWS // N_CORES  # 1024

MT = 128                  # m tile (PSUM partition dim)
NT = 512                  # n tile (one PSUM bank of fp32)
KT = 128                  # k tile (PE contraction dim)
N_MC = M_PER_CORE // MT   # 8
N_NC = OUT_F // NT        # 8
N_KC = IN_F // KT         # 32

N_F8KC = 8                # k-tiles [0, N_F8KC) run fp8 DoubleRow
N_DR = N_F8KC // 2        # DR matmuls per chain (each covers K=256)
N_BKC = N_KC - N_F8KC     # bf16 k-tiles per chain (kc in [N_F8KC, N_KC))

# bf16 rv: cols c = 3968 + 512*ncol - 128*kc for kc in [N_F8KC, N_KC)
RV_C0 = (N_KC - 1) * KT   # 3968
RV_F = RV_C0 - N_F8KC * KT + (N_NC - 1) * NT + NT  # 7552

# fp8 rv8[p, i, u] = vals[(C8 + 2p + i - u) mod 8191]; DR matmul t of
# output column block n0 streams rv8[:, :, s:s+512] with s = C8 - 256t + n0
C8 = 256 * (N_DR - 1)
RV8_F = C8 + (N_NC - 1) * NT + NT  # 4864

N_WARMUP_MM = 10          # dummy matmuls to lift the HAM clock gate

_COMPILED = None


def _legalize_single_wait(nc):
    """This walrus build encodes at most one sync-wait per instruction;
    move extra waits onto carrier NoOps on the same engine."""
    for f in nc.m.functions:
        for blk in f.blocks:
            insts = blk.instructions
            new = []
            changed = False
            for inst in insts:
                si = inst.sync_info
                if si is not None and si.on_wait is not None and len(si.on_wait) > 1:
                    waits = list(si.on_wait)
                    for w in waits[:-1]:
                        nop = mybir.InstNoOp(name=f"I-waitsplit-{nc.next_id()}")
                        nop.engine = inst.engine
                        nop.sync_info = bass_rust.SyncInfo(on_wait=[w], on_update=[])
                        new.append(nop)
                    inst.sync_info = bass_rust.SyncInfo(
                        on_wait=[waits[-1]], on_update=si.on_update
                    )
                    changed = True
                new.append(inst)
            if changed:
                blk.instructions = new


def build_nc():
    f32 = mybir.dt.float32
    bf16 = mybir.dt.bfloat16
    fp8 = mybir.dt.float8e4
    nc = bass.Bass()
    xT = nc.dram_tensor("xT", [IN_F, M_PER_CORE], bf16, kind="ExternalInput")
    xt8d = nc.dram_tensor(
        "xt8", [128, N_DR, 2, M_PER_CORE], fp8, kind="ExternalInput"
    )
    rv = nc.dram_tensor("rv", [128, RV_F], bf16, kind="ExternalInput")
    rv8d = nc.dram_tensor("rv8", [128, 2, RV8_F], fp8, kind="ExternalInput")
    bias_rep = nc.dram_tensor("bias_rep", [128, OUT_F], bf16, kind="ExternalInput")
    y = nc.dram_tensor("y", [M_PER_CORE, OUT_F], bf16, kind="ExternalOutput")

    # [128, N_BKC, M]; position q holds k-chunk kc = q + N_F8KC
    xT_r = xT.rearrange("(kc p) m -> p kc m", p=128)[:, N_F8KC:, :]

    with tile.TileContext(nc) as tc:
        with (
            tc.tile_pool(name="const", bufs=1) as cpool,
            tc.tile_pool(name="xp", bufs=3) as xpool,
            tc.tile_pool(name="op", bufs=8) as opool,
            tc.tile_pool(name="pp", bufs=8, space="PSUM") as ppool,
        ):
            # PE warm-up: the HAM clock gate keeps the PE at 1.2 GHz until
            # it has been busy ~3.4us. Run dummy matmuls on a memset tile
            # during the initial DMA wait so the clock is (mostly) up when
            # real matmuls start. The junk PSUM tile shares the 8-bank
            # 'acc' rotation; every real chain's start=True clear makes
            # that safe.
            junk_sb = cpool.tile([128, MT], bf16, name="junk_sb")
            nc.vector.memset(junk_sb, 0.0)
            junk_ps = ppool.tile([MT, MT], f32, tag="acc", name="junk_ps")
            for _ in range(N_WARMUP_MM):
                nc.tensor.matmul(junk_ps, junk_sb, junk_sb, start=True, stop=True)

            # Startup-critical path: matmul #0 (ncol=0, kc=31) needs only rv
            # cols [0,512) (128 KB) and xt chunk 31 (32 KB); the first chain
            # then consumes one more xt chunk and 128 more rv cols per
            # (bf16) step, reaching its two fp8 DR matmuls ~6 us in and the
            # first drain (bias) after that. Issue DMAs in consumption order.
            xt_first = xpool.tile([128, N_BKC, MT], bf16, tag="xt")
            xt8_first = xpool.tile([128, N_DR, 2, MT], fp8, tag="xt8")
            rv_sb = cpool.tile([128, RV_F], bf16)
            rv8_sb = cpool.tile([128, 2, RV8_F], fp8)
            bias_sb = cpool.tile([128, OUT_F], bf16)

            def rv_load(c0, c1):
                nc.sync.dma_start(out=rv_sb[:, c0:c1], in_=rv[:, c0:c1])

            def rv8_load(c0, c1):
                nc.sync.dma_start(
                    out=rv8_sb[:, :, c0:c1], in_=rv8d[:, :, c0:c1]
                )

            def xt_load(q0, q1):
                nc.sync.dma_start(
                    out=xt_first[:, q0:q1, :], in_=xT_r[:, q0:q1, 0:MT]
                )

            rv_load(0, 512)
            xt_load(N_BKC - 1, N_BKC)
            xt_load(N_BKC - 4, N_BKC - 1)
            rv_load(512, 1536)
            xt_load(N_BKC - 8, N_BKC - 4)
            rv_load(1536, 2560)
            xt_load(N_BKC - 16, N_BKC - 8)
            rv8_load(0, 1280)
            nc.sync.dma_start(
                out=xt8_first, in_=xt8d[:, :, :, 0:MT]
            )
            rv_load(2560, 4096)
            xt_load(0, N_BKC - 16)
            rv_load(4096, 5632)
            rv_load(5632, RV_F)
            rv8_load(1280, 2816)
            nc.sync.dma_start(out=bias_sb, in_=bias_rep[:, :])
            rv8_load(2816, RV8_F)

            for mc in range(N_MC):
                m0 = mc * MT
                if mc == 0:
                    xt_sb = xt_first
                    xt8_sb = xt8_first
                else:
                    xt_sb = xpool.tile([128, N_BKC, MT], bf16, tag="xt")
                    nc.sync.dma_start(out=xt_sb, in_=xT_r[:, :, m0 : m0 + MT])
                    xt8_sb = xpool.tile([128, N_DR, 2, MT], fp8, tag="xt8")
                    nc.sync.dma_start(
                        out=xt8_sb, in_=xt8d[:, :, :, m0 : m0 + MT]
                    )
                for ncol in range(N_NC):
                    n0 = ncol * NT
                    acc = ppool.tile([MT, NT], f32, tag="acc", name="acc")
                    for kk, kc in enumerate(reversed(range(N_F8KC, N_KC))):
                        c = RV_C0 + n0 - kc * KT
                        nc.tensor.matmul(
                            acc,
                            xt_sb[:, kc - N_F8KC, :],
                            rv_sb[:, c : c + NT],
                            start=(kk == 0),
                            stop=False,
                        )
                    for t in range(N_DR):
                        s = C8 - 256 * t + n0
                        nc.tensor.matmul(
                            acc,
                            xt8_sb[:, t],
                            rv8_sb[:, :, s : s + NT],
                            start=False,
                            stop=(t == N_DR - 1),
                            perf_mode=mybir.MatmulPerfMode.DoubleRow,
                        )
                    out_sb = opool.tile([MT, NT], bf16, tag="out")
                    nc.vector.tensor_add(
                        out_sb, acc, bias_sb[:, n0 : n0 + NT]
                    )
                    nc.sync.dma_start(
                        out=y[m0 : m0 + MT, n0 : n0 + NT], in_=out_sb
                    )
    _legalize_single_wait(nc)
    return nc


def _prep_shared(diagonals, bias):
    vals = np.concatenate([diagonals[OUT_F - 1 :], diagonals[: OUT_F - 1]])
    p = np.arange(128)[:, None]
    u = np.arange(RV_F)[None, :]
    rv = np.ascontiguousarray(vals[(RV_C0 + p - u) % NVALS].astype(BF16))
    p3 = np.arange(128)[:, None, None]
    i3 = np.arange(2)[None, :, None]
    u3 = np.arange(RV8_F)[None, None, :]
    rv8 = np.ascontiguousarray(
        vals[(C8 + 2 * p3 + i3 - u3) % NVALS].astype(E4M3)
    )
    bias_rep = np.ascontiguousarray(
        np.broadcast_to(bias.astype(BF16), (128, OUT_F))
    )
    return rv, rv8, bias_rep


def _prep_in_maps(x, diagonals, bias):
    """Host-side layout prep: per-core input dicts for run_bass_kernel_spmd."""
    x = np.asarray(x, dtype=np.float32)
    diagonals = np.asarray(diagonals, dtype=np.float32)
    bias = np.asarray(bias, dtype=np.float32)

    rv, rv8, bias_rep = _prep_shared(diagonals, bias)
    x2 = x.reshape(ROWS, IN_F)
    x2t = x2.T  # [IN_F, ROWS] view
    xb = x2t.astype(BF16)
    # xt8[ki, t, ko, m] = x2t[k, m] quantized e4m3, k = 256t + 2ki + ko
    ki = np.arange(128)[:, None, None]
    t = np.arange(N_DR)[None, :, None]
    ko = np.arange(2)[None, None, :]
    kidx = 256 * t + 2 * ki + ko  # [128, N_DR, 2]
    x8 = x2t[kidx, :].astype(E4M3)  # [128, N_DR, 2, ROWS]
    in_maps = []
    for c in range(N_CORES):
        msl = slice(c * M_PER_CORE, (c + 1) * M_PER_CORE)
        in_maps.append(
            {
                "xT": np.ascontiguousarray(xb[:, msl]),
                "xt8": np.ascontiguousarray(x8[:, :, :, msl]),
                "rv": rv,
                "rv8": rv8,
                "bias_rep": bias_rep,
            }
        )
    return in_maps


def kernel(x, diagonals, bias):
    global _COMPILED
    if _COMPILED is None:
        _COMPILED = build_nc()
    nc = _COMPILED

    in_maps = _prep_in_maps(x, diagonals, bias)
    res = run_bass_kernel_spmd(nc, in_maps, core_ids=list(range(N_CORES)))
    y = np.concatenate(
        [res.results[c]["y"].astype(np.float32) for c in range(N_CORES)],
        axis=0,
    )
    return y.reshape(B, S, OUT_F)
